# revision 20
# baseline (speedup 1.0000x reference)
"""CrossNet kernel for Trainium2, data-parallel over 8 NeuronCores.

Reference computation (per layer l = 0..3):
    s_l  = xl . W[l]                (per-row scalar)
    xl  <- x0 * s_l + b[l] + xl

Key algebraic collapse: xl always stays in the affine form
    xl_l = x0 * alpha_l + beta_l
with alpha_l a per-row scalar and beta_l a per-layer constant vector:
    alpha_0 = 1,  beta_0 = 0
    s_l       = alpha_l * p_l + q_l,   p_l = x0 . W[l]  (per-row),
                                       q_l = beta_l . W[l]  (host scalar)
    alpha_{l+1} = alpha_l * (1 + p_l) + q_l
    beta_{l+1}  = beta_l + b[l]
so the whole network needs just one skinny matmul P = x0 @ W^T, a
4-step per-row recurrence, and one fused output pass
    out = x0 * alpha_4 + beta_4.

v7 over the 74us v5 baseline:
  * x is cast to fp16 on the HOST and uploaded as fp16 - input HBM
    traffic halves (8.4 -> 4.2 MB/core), moving the DMA roofline from
    ~47us to ~35us.  fp16 keeps ~6e-4 rel err, far under the 2e-2 gate.
  * fp16 PE ops: transposes + contraction are single-pass (fp32
    matmuls are a 2x HI/LO pass pair on trn2).
  * output DMAs issue from the SP queue right after each DVE output
    op.  v6 had them on the ACT queue, where their stt-completion
    waits stalled the next group's PSUM->SBUF copies (~5us/group).
  * ramped group sizes (1,1,2,4,4,4 row-tiles): the first store
    issues after one 128-row tile instead of after 512 rows, so the
    output stream starts ~8us earlier and the store drain hides.
  * the +1 seed for the recurrence is folded into the ACT PSUM->SBUF
    copy of PT (scalar.add) - no ones-matmul.
"""

import numpy as np
import ml_dtypes
_np_bf16 = ml_dtypes.bfloat16

import concourse.bacc as bacc
import concourse.bass as bass
import concourse.tile as tile
from concourse import mybir
from concourse.bass_utils import run_bass_kernel_spmd

BATCH = 16384
DIM = 1024
NUM_LAYERS = 4
NCORES = 8
SHARD = BATCH // NCORES  # 2048
P = 128
NT = SHARD // P          # 16 row-tiles per core
NCHUNK = DIM // P        # 8 contraction chunks
# (tile_start, n_tiles) per contraction group: small groups first so the
# first output store issues early, 4-tile groups in steady state.
GROUPS = [(0, 1), (1, 1), (2, 2), (4, 4), (8, 4), (12, 4)]

_F32 = mybir.dt.float32
_F16 = mybir.dt.float16
_BF16 = mybir.dt.bfloat16

_cached_nc = None


def _build_program():
    nc = bacc.Bacc(None)

    x = nc.declare_dram_parameter("x", [SHARD, DIM], _BF16, isOutput=False)
    wt = nc.declare_dram_parameter("wt", [P, NCHUNK * NUM_LAYERS], _BF16, isOutput=False)
    qrow = nc.declare_dram_parameter("qrow", [1, NUM_LAYERS], _F32, isOutput=False)
    beta4 = nc.declare_dram_parameter("beta4", [1, DIM], _BF16, isOutput=False)
    id128 = nc.declare_dram_parameter("id128", [P, P], _BF16, isOutput=False)
    id4 = nc.declare_dram_parameter("id4", [NUM_LAYERS, NUM_LAYERS], _F32, isOutput=False)
    out = nc.declare_dram_parameter("out", [SHARD, DIM], _F32, isOutput=True)

    # dimension-aligned DRAM views (tile dims [p, s, d] match the SBUF
    # tile) - a [s, p, d]-ordered view silently scrambles fp16 DMAs.
    xv = {sz: x.rearrange("(n s p) d -> n p s d", s=sz, p=P) for sz in (1, 2, 4)}
    out_t = out.rearrange("(n p) d -> n p d", p=P)

    def bcast(ap, n):
        # read a [1, F] DRAM row broadcast onto n partitions
        return bass.AP(tensor=ap.tensor, offset=ap.offset, ap=[[0, n]] + list(ap.ap[1:]))

    with (
        tile.TileContext(nc) as tc,
        tc.tile_pool(name="consts", bufs=1) as consts,
        tc.tile_pool(name="xs", bufs=len(GROUPS)) as xs,
        tc.tile_pool(name="xt2", bufs=3) as xt2p,
        tc.tile_pool(name="outs", bufs=12) as outs,
        tc.tile_pool(name="pts", bufs=3) as pts,
        tc.tile_pool(name="als", bufs=8) as als,
        tc.tile_pool(name="ps_xt", bufs=4, space="PSUM") as ps_xt,
        tc.tile_pool(name="ps_pt", bufs=2, space="PSUM") as ps_pt,
        tc.tile_pool(name="ps_p", bufs=2, space="PSUM") as ps_p,
    ):
        # All loads go up front on the SP HWDGE queue, ordered by first
        # use: X group 0 + id128 gate the first transposes, wt gates the
        # first contraction, qrow/beta4 (slow 128-descriptor broadcasts)
        # are needed only by the first DVE tail a few us in.
        def load_x(g):
            t0, sz = GROUPS[g]
            X = xs.tile([P, sz, DIM], _BF16, tag=f"X{sz}")
            nc.sync.dma_start(out=X, in_=xv[sz][t0 // sz])
            return X

        X_tiles = [None] * len(GROUPS)
        X_tiles[0] = load_x(0)
        id128_sb = consts.tile([P, P], _BF16)
        nc.sync.dma_start(out=id128_sb, in_=id128[:])
        # beta4 early: it gates the first DVE output op, which gates the
        # first store - the whole output stream shifts with it.
        beta4_sb = consts.tile([P, DIM], _BF16)
        nc.sync.dma_start(out=beta4_sb, in_=bcast(beta4[:], P))
        X_tiles[1] = load_x(1)
        wt_sb = consts.tile([P, NCHUNK * NUM_LAYERS], _BF16)
        nc.sync.dma_start(out=wt_sb, in_=wt[:])
        qrow_sb = consts.tile([P, NUM_LAYERS], _F32)
        nc.sync.dma_start(out=qrow_sb, in_=bcast(qrow[:], P))
        id4_sb = consts.tile([NUM_LAYERS, NUM_LAYERS], _F32)
        nc.sync.dma_start(out=id4_sb, in_=id4[:])
        X_tiles[2] = load_x(2)
        X_tiles[3] = load_x(3)
        X_tiles[4] = load_x(4)
        X_tiles[5] = load_x(5)

        for g, (t0, sz) in enumerate(GROUPS):
            NB = sz * P
            X = X_tiles[g]
            # ---- transpose sz sub-tiles into XT2 --------------------
            # XT2[d_in_chunk, c, j*128+b] = X[b, j, c*128+d]
            XT2 = xt2p.tile([P, NCHUNK, NB], _BF16, tag="XT2")
            for j in range(sz):
                Xs = X[:, j, :]
                XT_ps = ps_xt.tile([P, DIM], _BF16)
                for c in range(NCHUNK):
                    nc.tensor.transpose(
                        XT_ps[:, c * P:(c + 1) * P], Xs[:, c * P:(c + 1) * P], id128_sb
                    )
                nc.scalar.copy(
                    XT2[:, :, j * P:(j + 1) * P],
                    XT_ps.rearrange("p (c b) -> p c b", c=NCHUNK),
                )

            # ---- PT[l, n] = sum_d W[l, d] * XT2[d, :, n] ------------
            PT_ps = ps_pt.tile([NUM_LAYERS, NB], _F32, tag="PT")
            for c in range(NCHUNK):
                nc.tensor.matmul(
                    PT_ps,
                    wt_sb[:, c * NUM_LAYERS:(c + 1) * NUM_LAYERS],
                    XT2[:, c, :],
                    start=(c == 0),
                    stop=(c == NCHUNK - 1),
                )
            # +1 for the recurrence fused with the PSUM->SBUF copy; on
            # DVE (cheap: 4 partitions) so the ACT queue stays pure
            # copies - an ACT-resident PTadd chained PTadd(g) ->
            # copies(g+1) -> contraction(g+1) into a 6.7us/group cycle.
            PT = pts.tile([NUM_LAYERS, NB], _F32, tag="PT")
            nc.vector.tensor_scalar_add(PT, PT_ps, 1.0)

            for j in range(sz):
                Xs = X[:, j, :]
                # back to [b, l] layout for the per-row recurrence
                P_ps = ps_p.tile([P, NUM_LAYERS], _F32)
                nc.tensor.transpose(P_ps, PT[:, j * P:(j + 1) * P], id4_sb)

                # alpha_{l+1} = alpha_l * (1 + p_l) + q_l, alpha_0 = 1
                AL = als.tile([P, NUM_LAYERS], _F32)
                nc.vector.tensor_tensor_scan(
                    AL, P_ps, qrow_sb, 1.0, mybir.AluOpType.mult, mybir.AluOpType.add
                )

                # out = x0 * alpha_4 + beta_4, fused in one DVE op.
                # All-fp16 tensor operands for 2-port DVE throughput;
                # the SWDGE store casts fp16 -> f32 on the way out (only
                # gpsimd DMAs can cast), off the busy HWDGE queues.
                # out = x0 * alpha_4 + beta_4, fused in one DVE op.
                # O is bf16 (fp16 overflows: |out| reaches ~4e7); all
                # 16-bit tensor operands keep DVE in 2-port mode, and
                # the SWDGE store casts bf16 -> f32 on the way out
                # (only gpsimd DMAs can cast), off the HWDGE queues.
                O = outs.tile([P, DIM], _BF16)
                nc.vector.scalar_tensor_tensor(
                    O, Xs, AL[:, NUM_LAYERS - 1:NUM_LAYERS],
                    beta4_sb, mybir.AluOpType.mult, mybir.AluOpType.add,
                )
                nc.gpsimd.dma_start(out=out_t[t0 + j], in_=O)

    nc.compile()
    return nc


def _host_constants(W, b):
    W64 = W.astype(np.float64)
    b64 = b.astype(np.float64)
    q = np.zeros(NUM_LAYERS, dtype=np.float64)
    beta = np.zeros(DIM, dtype=np.float64)
    for l in range(NUM_LAYERS):
        q[l] = beta @ W64[l]
        beta += b64[l]
    # wt[k, c*4 + l] = W[l, c*128 + k]
    wt = np.ascontiguousarray(
        W.T.reshape(NCHUNK, P, NUM_LAYERS).transpose(1, 0, 2).reshape(P, NCHUNK * NUM_LAYERS)
    ).astype(_np_bf16)
    qrow = q.astype(np.float32).reshape(1, NUM_LAYERS)
    beta4 = beta.astype(_np_bf16).reshape(1, DIM)
    id128 = np.eye(P, dtype=_np_bf16)
    id4 = np.eye(NUM_LAYERS, dtype=np.float32)
    return wt, qrow, beta4, id128, id4


def _run(x0, W, b, trace=False):
    global _cached_nc
    if _cached_nc is None:
        _cached_nc = _build_program()
    nc = _cached_nc

    x16 = np.ascontiguousarray(np.asarray(x0, dtype=np.float32).astype(_np_bf16))
    wt, qrow, beta4, id128, id4 = _host_constants(
        np.asarray(W, dtype=np.float32), np.asarray(b, dtype=np.float32)
    )
    shards = x16.reshape(NCORES, SHARD, DIM)
    in_maps = [
        {"x": shards[i], "wt": wt, "qrow": qrow, "beta4": beta4,
         "id128": id128, "id4": id4}
        for i in range(NCORES)
    ]
    res = run_bass_kernel_spmd(nc, in_maps, list(range(NCORES)), trace=trace)
    out = np.concatenate([res.results[i]["out"] for i in range(NCORES)], axis=0)
    return out, res


def kernel(x0, W, b):
    out, _ = _run(x0, W, b, trace=False)
    return out


def _register_ntff_hook():
    """The container's antenv stub lacks axon_hooks; replicate the boot-time
    ctypes NTFF hook (see trn_boot._ntff_profile_via_ctypes) so trace=True
    can capture HW profiles."""
    import sys
    import types
    import ctypes
    import contextlib

    if "antenv.axon_hooks" in sys.modules:
        return
    so_path = "/opt/axon/libaxon_pjrt.so"
    lib = ctypes.CDLL(so_path)
    if not hasattr(lib, "axon_start_nrt_profile"):
        return
    lib.axon_start_nrt_profile.argtypes = [
        ctypes.POINTER(ctypes.c_int64),
        ctypes.c_size_t,
    ]
    lib.axon_start_nrt_profile.restype = ctypes.c_int64
    lib.axon_stop_nrt_profile.argtypes = [ctypes.c_char_p]
    lib.axon_stop_nrt_profile.restype = ctypes.c_int64

    @contextlib.contextmanager
    def _hook(output_dir, device_ids):
        import jax

        jax.devices()
        if device_ids:
            ids = (ctypes.c_int64 * len(device_ids))(*device_ids)
            rc = lib.axon_start_nrt_profile(ids, len(device_ids))
        else:
            rc = lib.axon_start_nrt_profile(None, 0)
        if rc != 0:
            raise RuntimeError(f"axon_start_nrt_profile rc={rc}")
        try:
            yield
        finally:
            n = lib.axon_stop_nrt_profile(str(output_dir).encode())
            print(f"ntff profile: {n} file(s) written to {output_dir}")

    mod = types.ModuleType("antenv.axon_hooks")
    mod.get_axon_ntff_profile_hook = lambda: _hook
    mod.set_axon_ntff_profile_hook = lambda h: None
    sys.modules["antenv.axon_hooks"] = mod


def kernel_timed(x0, W, b):
    _register_ntff_hook()
    out, res = _run(x0, W, b, trace=True)
    return out, res
